# revision 43
# baseline (speedup 1.0000x reference)
"""Bass/Tile TRN2 kernel for the coverage-attention module.

Math (per batch b):
    enc_feature = enc_output[b] @ W_h.T                    [S, H]
    dec_feature = W_s @ dec_hidden[b] + b_s                [H]
    feat        = enc_feature + dec_feature + cov[b,:,None]*W_c[:,0]
    e_t         = tanh(feat) @ V[0]                        [S]
    dist        = softmax(e_t + mask[b])                   [S]
    ctx         = dist @ enc_output[b]                     [H]
    new_cov     = cov[b] + dist

Strategy: data-parallel over batch across 8 cores (4 batches/core),
weights replicated.  Matmuls run in bf16 (fp32 matmul is 4x slower on
the PE; bf16 with K=1024 fp32 accumulation keeps relative error around
2.4e-3, well under the 2e-2 gate).  Per batch the sequence runs in 16
s-tiles of 128 rows, single pass over enc_output:
  - The h-contraction ("transposed") layout of each enc tile is packed
    host-side (pure layout prep, like the weight transposes) and DMA'd
    directly, so the PE spends no cycles transposing.
  - PE: 16 N=512 main matmuls per s-tile; one zero-padded K=128 rank
    update per h_out half folds dec_feature + coverage*W_c + b_s into
    the same PSUM accumulation (zero-padding keeps the fast FWL weight
    path: ~225ns vs ~317ns per matmul at K<32); the context vector
    accumulates via [K=128,M=1]x[K=128,N=512] matmuls against exp(e_t).
  - ACT: tanh (PSUM->SBUF, fp32) and exp (the padding mask enters as
    the per-partition bias of the exp activation).
  - DVE: V-dot as tensor_mul + reduce_sum (fp32).
Softmax is computed without max subtraction (|e_t| <= sum|V| <= 16 so
exp cannot overflow in fp32), which makes it single-pass: p-columns
feed the context matmul two iterations behind the main loop;
normalization by 1/total happens at the batch tail, where a
[128,nt]-ones fp32 matmul replicates the partition-sum total across
partitions (exact, no extra rounding).

Scheduling: dependency-free junk matmuls warm the PE clock gate (HAM)
during the initial weight DMAs; the first encT tile is split across 4
DMA queues; batch b's last epilogue + tail are emitted after batch
b+1's prefetch/setup so the PE fills the softmax-chain wait; the
whole kernel is straight-line code (no loops) scheduled by Tile.

This walrus build rejects >1 sync wait per instruction (2 for
EventSemaphore): _cap_waits() post-processes the scheduled program,
spilling excess waits onto single-wait EventSemaphore instructions on
the same engine.
"""

import numpy as np

import concourse.bass as bass
import concourse.mybir as mybir
from concourse.masks import make_identity
from concourse.tile import TileContext

F32 = mybir.dt.float32
BF16 = mybir.dt.bfloat16
AF = mybir.ActivationFunctionType
ALU = mybir.AluOpType

N_CORES = 8
B, S, H = 32, 2048, 1024
P = 128


def _cap_waits(nc):
    """Cap sync waits per instruction for this container's walrus build.

    The walrus here rejects instructions carrying more than 1 sync wait
    (2 for EventSemaphore) with "Too many sync wait commands", while the
    Tile scheduler freely attaches one wait per dependency.  Move excess
    waits onto InstEventSemaphore instructions inserted just before the
    owner on the same engine — the engine executes its stream in order,
    so the conjunction of waits is preserved.
    """
    import bass_rust

    for f in nc.m.functions:
        for blk in f.blocks:
            insts = blk.instructions
            new_list = []
            changed = False
            for ins in insts:
                si = ins.sync_info
                waits = list(si.on_wait) if si is not None and si.on_wait else []
                cap = 2 if isinstance(ins, mybir.InstEventSemaphore) else 1
                if len(waits) > cap:
                    changed = True
                    extra, keep = waits[:-cap], waits[-cap:]
                    for j in range(0, len(extra), 2):
                        ev = mybir.InstEventSemaphore(
                            name=f"I-{nc.next_id()}",
                            engine=ins.engine,
                            ins=[],
                            outs=[],
                            sync_info=bass_rust.SyncInfo(
                                on_wait=extra[j : j + 2], on_update=[]
                            ),
                        )
                        new_list.append(ev)
                    ins.sync_info = bass_rust.SyncInfo(
                        on_wait=keep,
                        on_update=list(si.on_update) if si.on_update else [],
                    )
                new_list.append(ins)
            if changed:
                blk.instructions = new_list


def build(nc, bpc, s, h):
    """Emit the per-core program. bpc batches of [s, h] per core."""
    nt = s // P          # s-tiles per batch
    nck = h // P         # h_in chunks of 128
    nho = h // 512       # h_out chunks of 512
    nr = h // P          # rows per batch in the dec reshape

    enc = nc.dram_tensor("enc", [bpc, s, h], BF16, kind="ExternalInput").ap()
    # per-s-tile transposed enc: encTt[b, t, p, c*128+s'] = enc[b, t*128+s', c*128+p]
    encTt = nc.dram_tensor(
        "encTt", [bpc, s // P, P, h], BF16, kind="ExternalInput"
    ).ap()
    # dec_hidden repacked host-side into columns: deccol[p, b*nr+c] = dec[b, c*128+p]
    deccol = nc.dram_tensor("deccol", [P, bpc * nr], BF16, kind="ExternalInput").ap()
    mask = nc.dram_tensor("mask", [bpc, s], F32, kind="ExternalInput").ap()
    cov = nc.dram_tensor("cov", [bpc, s], F32, kind="ExternalInput").ap()
    nk6 = bpc + 2  # rank of the fused update: bpc dec rows + W_c + b_s
    # lc6[b] = [onehot(b); cov[b]; ones] — stationary side of the rank-(bpc+2)
    # update that folds dec_feature + coverage*W_c + b_s into the feat PSUM
    lc6 = nc.dram_tensor("lc6", [bpc, nk6, s], BF16, kind="ExternalInput").ap()
    # wb2 = [W_c[:,0]; b_s] — moving side rows 1-2 of the rank-3 update
    wb2 = nc.dram_tensor("wb2", [2, h], BF16, kind="ExternalInput").ap()
    whT = nc.dram_tensor("whT", [h, h], BF16, kind="ExternalInput").ap()
    wsT = nc.dram_tensor("wsT", [h, h], BF16, kind="ExternalInput").ap()
    v = nc.dram_tensor("v", [h], F32, kind="ExternalInput").ap()

    octx = nc.dram_tensor("octx", [bpc, h], F32, kind="ExternalOutput").ap()
    odist = nc.dram_tensor("odist", [bpc, s], F32, kind="ExternalOutput").ap()
    oncov = nc.dram_tensor("oncov", [bpc, s], F32, kind="ExternalOutput").ap()

    with TileContext(nc) as tc:
        with (
            tc.tile_pool(name="const", bufs=1) as cpool,
            tc.tile_pool(name="enc", bufs=6) as epool,
            tc.tile_pool(name="encT", bufs=4) as etpool,
            tc.tile_pool(name="tanh", bufs=3) as thpool,
            tc.tile_pool(name="scratch", bufs=3) as scpool,
            tc.tile_pool(name="small", bufs=2) as smpool,
            tc.tile_pool(name="psf", bufs=2, space="PSUM") as psf,
            tc.tile_pool(name="pst", bufs=2, space="PSUM") as pst,
            tc.tile_pool(name="psc", bufs=1, space="PSUM") as psc,
        ):
            # ---- constants ----
            ident_b = cpool.tile([P, P], BF16)
            make_identity(nc, ident_b)
            ident_f = cpool.tile([P, P], F32)
            make_identity(nc, ident_f)
            # HAM warmup: dependency-free junk matmuls keep the PE busy
            # from t~1us through the initial weight DMAs so the clock gate
            # is at 8/8 when real work starts (values are never read)
            junk = cpool.tile([P, P], BF16)
            nc.vector.memset(junk, 0.0)
            for j in range(48):
                jp = pst.tile([P, P], F32, tag="tp", name="jp")
                nc.tensor.matmul(
                    jp, lhsT=junk, rhs=junk, start=True, stop=True,
                    skip_group_check=True,
                )

            def setup(b, st=None):
                st = st if st is not None else {}
                nc.sync.dma_start(out=lcp[0:nk6, :], in_=lc6[b])
                cov_nat = smpool.tile([nt, P], F32, name="cov_nat")
                nc.sync.dma_start(
                    out=cov_nat, in_=cov[b].rearrange("(t p) -> t p", p=P)
                )
                mask_nat = smpool.tile([nt, P], F32, name="mask_nat")
                nc.sync.dma_start(
                    out=mask_nat, in_=mask[b].rearrange("(t p) -> t p", p=P)
                )
                mt_ps = pst.tile([P, 256], F32, tag="tp", name="mt_ps")
                nc.tensor.transpose(mt_ps[:, :nt], mask_nat, ident_f[:nt, :nt])
                maskT = smpool.tile([P, nt], F32, name="maskT")
                nc.vector.tensor_copy(maskT, mt_ps[:, :nt])

                st["cov_nat"] = cov_nat
                st["maskT"] = maskT
                st["p_all"] = smpool.tile([P, nt], BF16, name="p_all")
                st["ctx_ps"] = psc.tile([1, h], F32, name="ctx_ps")
                st.setdefault("enc_t", [None] * nt)
                st["et"] = [None] * nt
                return st

            def epilogue(b, st, t):
                # exp(e_t + mask) -> p column t; then accumulate ctx.
                nc.scalar.activation(
                    st["p_all"][:, t : t + 1],
                    st["et"][t],
                    AF.Exp,
                    bias=st["maskT"][:, t : t + 1],
                    scale=1.0,
                )
                for ho in range(nho):
                    hsl = slice(ho * 512, (ho + 1) * 512)
                    nc.tensor.matmul(
                        st["ctx_ps"][0:1, hsl],
                        lhsT=st["p_all"][:, t : t + 1],
                        rhs=st["enc_t"][t][:, hsl],
                        start=(t == 0),
                        stop=(t == nt - 1),
                        skip_group_check=True,
                    )

            def load_enc(b, st, t):
                st["enc_t"][t] = epool.tile([P, h], BF16, tag="enc_t", name="enc_t")
                nc.scalar.dma_start(
                    out=st["enc_t"][t], in_=enc[b, t * P : (t + 1) * P, :]
                )

            def load_encT(b, st, t):
                # h-contraction layout, prepacked host-side
                encT = etpool.tile([P, h], BF16, tag="encT", name="encT")
                nc.sync.dma_start(out=encT, in_=encTt[b, t])
                st["encT"][t] = encT

            def vdot(b, st, t):
                # e_t = sum_h tanh(feat) * V — runs one iteration behind so
                # the encT copies always lead the DVE stream.
                prod = scpool.tile([P, h], F32, tag="prod", name="prod")
                st["et"][t] = smpool.tile([P, 1], F32, tag="et", name="et")
                nc.vector.tensor_mul(prod, st["tanh"][t], v_bc)
                nc.vector.reduce_sum(st["et"][t], prod, axis=mybir.AxisListType.X)

            def sloop(b, st):
                for t in range(nt):
                    if t + 2 < nt:
                        load_enc(b, st, t + 2)
                        load_encT(b, st, t + 2)
                    if t > 0:
                        vdot(b, st, t - 1)
                    feat = psf.tile([P, h], F32, tag="feat", name="feat")
                    for ho in range(nho):
                        hsl = slice(ho * 512, (ho + 1) * 512)
                        for c in range(nck):
                            nc.tensor.matmul(
                                feat[:, hsl],
                                lhsT=st["encT"][t][:, c * P : (c + 1) * P],
                                rhs=w_c[c][:, hsl],
                                start=(c == 0),
                                stop=False,
                            )
                        nc.tensor.matmul(
                            feat[:, hsl],
                            lhsT=lcp[:, t * P : (t + 1) * P],
                            rhs=rxp[:, hsl],
                            start=False,
                            stop=True,
                        )
                    if t > 1:
                        epilogue(b, st, t - 2)
                    tanh_t = thpool.tile([P, h], F32, tag="tanh", name="tanh_t")
                    nc.scalar.activation(tanh_t, feat, AF.Tanh)
                    st["tanh"][t] = tanh_t

            def finish(b, st):
                vdot(b, st, nt - 1)
                epilogue(b, st, nt - 2)
                epilogue(b, st, nt - 1)

            def tail(b, st):
                # total, normalize, outputs
                prow = smpool.tile([P, 1], F32, tag="prow", name="prow")
                nc.vector.reduce_sum(prow, st["p_all"], axis=mybir.AxisListType.X)
                # [nt,1] of the total: ones[128,nt].T @ prow replicates the
                # partition-sum into every output partition (fp32, exact).
                tot_ps = pst.tile([P, 512], F32, tag="tp", name="tot_ps")
                nc.tensor.matmul(
                    tot_ps[:nt, 0:1],
                    lhsT=nt_ones,
                    rhs=prow,
                    start=True,
                    stop=True,
                    skip_group_check=True,
                )
                inv128 = smpool.tile([nt, 1], F32, tag="inv128", name="inv128")
                nc.vector.reciprocal(inv128, tot_ps[:nt, 0:1])

                pt_ps = pst.tile([P, P], BF16, tag="tp", name="pt_ps")
                nc.tensor.transpose(pt_ps[:nt, :P], st["p_all"], ident_b)
                dist_nat = smpool.tile([nt, P], F32, tag="dist_nat", name="dist_nat")
                nc.vector.tensor_scalar_mul(
                    dist_nat, pt_ps[:nt, :P], inv128[:nt, :]
                )
                ncov_nat = smpool.tile([nt, P], F32, tag="ncov_nat", name="ncov_nat")
                nc.vector.tensor_add(ncov_nat, st["cov_nat"], dist_nat)
                ctx_sb = smpool.tile([1, h], F32, tag="ctx_sb", name="ctx_sb")
                nc.vector.tensor_scalar_mul(ctx_sb, st["ctx_ps"], inv128[0:1, :])

                nc.scalar.dma_start(
                    out=odist[b].rearrange("(t p) -> t p", p=P), in_=dist_nat
                )
                nc.scalar.dma_start(
                    out=oncov[b].rearrange("(t p) -> t p", p=P), in_=ncov_nat
                )
                nc.scalar.dma_start(
                    out=octx[b].rearrange("(o n) -> o n", o=1), in_=ctx_sb
                )

            # zero-padded stationary side of the fused update, rows
            # 0..nk6-1 refreshed per batch
            lcp = cpool.tile([P, s], BF16)
            nc.vector.memset(lcp, 0.0)

            # ---- first-wave DMAs: the tensors the PE needs first (first
            # encT tile, weights, dec columns) get the first queue slots ----
            st0 = {"enc_t": [None] * nt, "encT": [None] * nt, "tanh": [None] * nt}
            encT00 = etpool.tile([P, h], BF16, tag="encT", name="encT00")
            for q in range(4):
                qs = slice(q * (h // 4), (q + 1) * (h // 4))
                nc.sync.dma_start(out=encT00[:, qs], in_=encTt[0, 0][:, qs])
            st0["encT"][0] = encT00
            # per-chunk weight tiles so the first matmuls only wait on the
            # first 128 rows, not the whole 2 MB weight DMA
            w_c = []
            for c in range(nck):
                wt = cpool.tile([P, h], BF16, name=f"w_c{c}")
                nc.sync.dma_start(out=wt, in_=whT[c * P : (c + 1) * P, :])
                w_c.append(wt)
            encT01 = etpool.tile([P, h], BF16, tag="encT", name="encT01")
            for q in range(2):
                qs = slice(q * (h // 2), (q + 1) * (h // 2))
                nc.sync.dma_start(out=encT01[:, qs], in_=encTt[0, 1][:, qs])
            st0["encT"][1] = encT01
            ws_c = []
            for c in range(nck):
                wt = cpool.tile([P, h], BF16, name=f"ws_c{c}")
                nc.sync.dma_start(out=wt, in_=wsT[c * P : (c + 1) * P, :])
                ws_c.append(wt)
            decall = cpool.tile([P, bpc * nr], BF16)
            nc.sync.dma_start(out=decall, in_=deccol)
            load_enc(0, st0, 0)
            load_enc(0, st0, 1)
            setup(0, st0)
            v_bc = cpool.tile([P, h], F32)
            nc.sync.dma_start(out=v_bc, in_=v.partition_broadcast(P))
            nt_ones = cpool.tile([P, nt], F32)
            nc.vector.memset(nt_ones, 1.0)

            # rxp = [dec_features(all 4 batches); W_c; b_s; zeros...] — the
            # moving side of the per-tile fused update, zero-padded to K=128
            # so the matmul takes the fast (FWL) weight-load path.  Rows 0-3
            # computed on device with one M=4 matmul group per h_out half.
            rxp = cpool.tile([P, h], BF16)
            nc.vector.memset(rxp, 0.0)
            nc.sync.dma_start(out=rxp[bpc : bpc + 2, :], in_=wb2)
            for ho in range(nho):
                hsl = slice(ho * 512, (ho + 1) * 512)
                dps = pst.tile([bpc, 512], F32, tag="tp", name="dps")
                for c in range(nck):
                    nc.tensor.matmul(
                        dps,
                        lhsT=decall[:, c::nr],
                        rhs=ws_c[c][:, hsl],
                        start=(c == 0),
                        stop=(c == nck - 1),
                    )
                nc.vector.tensor_copy(rxp[0:bpc, hsl], dps)
            # Batch-tail software pipeline: batch b's last epilogue and tail
            # are emitted after batch b+1's setup and first transposes, so
            # the PE fills the softmax-chain wait with useful work.
            states = {0: st0}
            for b in range(bpc):
                st = states[b]
                if b > 0:
                    finish(b - 1, states[b - 1])
                    tail(b - 1, states[b - 1])
                sloop(b, st)
                if b + 1 < bpc:
                    # next batch: enc prefetch + small-tensor DMAs
                    nx = {
                        "enc_t": [None] * nt,
                        "encT": [None] * nt,
                        "tanh": [None] * nt,
                    }
                    states[b + 1] = nx
                    load_encT(b + 1, nx, 0)
                    load_enc(b + 1, nx, 0)
                    load_encT(b + 1, nx, 1)
                    load_enc(b + 1, nx, 1)
                    setup(b + 1, nx)
            finish(bpc - 1, states[bpc - 1])
            tail(bpc - 1, states[bpc - 1])
    _cap_waits(nc)
    return nc


def make_nc(bpc=B // N_CORES, s=S, h=H):
    nc = bass.Bass("TRN2", target_bir_lowering=False, debug=False, num_devices=1)
    return build(nc, bpc, s, h)


def make_in_maps(enc_output, dec_hidden, enc_padding_mask, coverage, W_h, W_s, b_s, W_c, V):
    import ml_dtypes

    bf16 = ml_dtypes.bfloat16
    enc_output = np.asarray(enc_output, dtype=np.float32).astype(bf16)
    dec_hidden = np.asarray(dec_hidden, dtype=np.float32).astype(bf16)
    enc_padding_mask = np.ascontiguousarray(np.asarray(enc_padding_mask, dtype=np.float32))
    coverage = np.ascontiguousarray(np.asarray(coverage, dtype=np.float32))
    whT = np.ascontiguousarray(np.asarray(W_h, dtype=np.float32).T.astype(bf16))
    wsT = np.ascontiguousarray(np.asarray(W_s, dtype=np.float32).T.astype(bf16))
    bs = np.asarray(b_s, dtype=np.float32).astype(bf16)
    wc = np.asarray(W_c, dtype=np.float32)[:, 0].astype(bf16)
    v = np.ascontiguousarray(np.asarray(V, dtype=np.float32)[0])
    wb2 = np.ascontiguousarray(np.stack([wc, bs], axis=0))

    nb, s = coverage.shape
    h = v.shape[0]
    nr = h // P
    bpc = nb // N_CORES
    in_maps = []
    for i in range(N_CORES):
        sl = slice(i * bpc, (i + 1) * bpc)
        dec_i = dec_hidden[sl]  # [bpc, h]
        deccol = np.ascontiguousarray(
            dec_i.reshape(bpc, nr, P).transpose(2, 0, 1).reshape(P, bpc * nr)
        )
        lc6 = np.zeros((bpc, bpc + 2, s), dtype=bf16)
        for bb in range(bpc):
            lc6[bb, bb, :] = 1.0
        lc6[:, bpc, :] = coverage[sl].astype(bf16)
        lc6[:, bpc + 1, :] = 1.0
        in_maps.append(
            {
                "enc": np.ascontiguousarray(enc_output[sl]),
                "encTt": np.ascontiguousarray(
                    enc_output[sl]
                    .reshape(bpc, s // P, P, h // P, P)
                    .transpose(0, 1, 4, 3, 2)
                    .reshape(bpc, s // P, P, h)
                ),
                "deccol": deccol,
                "mask": enc_padding_mask[sl],
                "cov": coverage[sl],
                "lc6": np.ascontiguousarray(lc6),
                "wb2": wb2,
                "whT": whT,
                "wsT": wsT,
                "v": v,
            }
        )
    return in_maps, bpc


def run(in_maps, nc=None, **kw):
    from concourse.bass_utils import run_bass_kernel_spmd

    if nc is None:
        nc = make_nc()
    return run_bass_kernel_spmd(nc, in_maps, core_ids=list(range(N_CORES)), **kw)


def kernel(**inputs):
    in_maps, bpc = make_in_maps(**inputs)
    res = run(in_maps)
    ctx = np.concatenate([res.results[i]["octx"] for i in range(N_CORES)], axis=0)
    dist = np.concatenate([res.results[i]["odist"] for i in range(N_CORES)], axis=0)
    ncov = np.concatenate([res.results[i]["oncov"] for i in range(N_CORES)], axis=0)
    return ctx, dist, ncov


# revision 44
# speedup vs baseline: 1.0065x; 1.0065x over previous
"""Bass/Tile TRN2 kernel for the coverage-attention module.

Math (per batch b):
    enc_feature = enc_output[b] @ W_h.T                    [S, H]
    dec_feature = W_s @ dec_hidden[b] + b_s                [H]
    feat        = enc_feature + dec_feature + cov[b,:,None]*W_c[:,0]
    e_t         = tanh(feat) @ V[0]                        [S]
    dist        = softmax(e_t + mask[b])                   [S]
    ctx         = dist @ enc_output[b]                     [H]
    new_cov     = cov[b] + dist

Strategy: data-parallel over batch across 8 cores (4 batches/core),
weights replicated.  Matmuls run in bf16 (fp32 matmul is 4x slower on
the PE; bf16 with K=1024 fp32 accumulation keeps relative error around
2.4e-3, well under the 2e-2 gate).  Per batch the sequence runs in 16
s-tiles of 128 rows, single pass over enc_output:
  - The h-contraction ("transposed") layout of each enc tile is packed
    host-side (pure layout prep, like the weight transposes) and DMA'd
    directly, so the PE spends no cycles transposing.
  - PE: 16 N=512 main matmuls per s-tile; one zero-padded K=128 rank
    update per h_out half folds dec_feature + coverage*W_c + b_s into
    the same PSUM accumulation (zero-padding keeps the fast FWL weight
    path: ~225ns vs ~317ns per matmul at K<32); the context vector
    accumulates via [K=128,M=1]x[K=128,N=512] matmuls against exp(e_t).
  - ACT: tanh (PSUM->SBUF, fp32) and exp (the padding mask enters as
    the per-partition bias of the exp activation).
  - DVE: V-dot as tensor_mul + reduce_sum (fp32).
Softmax is computed without max subtraction (|e_t| <= sum|V| <= 16 so
exp cannot overflow in fp32), which makes it single-pass: p-columns
feed the context matmul two iterations behind the main loop;
normalization by 1/total happens at the batch tail, where a
[128,nt]-ones fp32 matmul replicates the partition-sum total across
partitions (exact, no extra rounding).

Scheduling: dependency-free junk matmuls warm the PE clock gate (HAM)
during the initial weight DMAs; the first encT tile is split across 4
DMA queues; batch b's last epilogue + tail are emitted after batch
b+1's prefetch/setup so the PE fills the softmax-chain wait; the
whole kernel is straight-line code (no loops) scheduled by Tile.

This walrus build rejects >1 sync wait per instruction (2 for
EventSemaphore): _cap_waits() post-processes the scheduled program,
spilling excess waits onto single-wait EventSemaphore instructions on
the same engine.
"""

import numpy as np

import concourse.bass as bass
import concourse.mybir as mybir
from concourse.masks import make_identity
from concourse.tile import TileContext

F32 = mybir.dt.float32
BF16 = mybir.dt.bfloat16
AF = mybir.ActivationFunctionType
ALU = mybir.AluOpType

N_CORES = 8
B, S, H = 32, 2048, 1024
P = 128


def _cap_waits(nc):
    """Cap sync waits per instruction for this container's walrus build.

    The walrus here rejects instructions carrying more than 1 sync wait
    (2 for EventSemaphore) with "Too many sync wait commands", while the
    Tile scheduler freely attaches one wait per dependency.  Move excess
    waits onto InstEventSemaphore instructions inserted just before the
    owner on the same engine — the engine executes its stream in order,
    so the conjunction of waits is preserved.
    """
    import bass_rust

    for f in nc.m.functions:
        for blk in f.blocks:
            insts = blk.instructions
            new_list = []
            changed = False
            for ins in insts:
                si = ins.sync_info
                waits = list(si.on_wait) if si is not None and si.on_wait else []
                cap = 2 if isinstance(ins, mybir.InstEventSemaphore) else 1
                if len(waits) > cap:
                    changed = True
                    extra, keep = waits[:-cap], waits[-cap:]
                    for j in range(0, len(extra), 2):
                        ev = mybir.InstEventSemaphore(
                            name=f"I-{nc.next_id()}",
                            engine=ins.engine,
                            ins=[],
                            outs=[],
                            sync_info=bass_rust.SyncInfo(
                                on_wait=extra[j : j + 2], on_update=[]
                            ),
                        )
                        new_list.append(ev)
                    ins.sync_info = bass_rust.SyncInfo(
                        on_wait=keep,
                        on_update=list(si.on_update) if si.on_update else [],
                    )
                new_list.append(ins)
            if changed:
                blk.instructions = new_list


def build(nc, bpc, s, h):
    """Emit the per-core program. bpc batches of [s, h] per core."""
    nt = s // P          # s-tiles per batch
    nck = h // P         # h_in chunks of 128
    nho = h // 512       # h_out chunks of 512
    nr = h // P          # rows per batch in the dec reshape

    enc = nc.dram_tensor("enc", [bpc, s, h], BF16, kind="ExternalInput").ap()
    # per-s-tile transposed enc: encTt[b, t, p, c*128+s'] = enc[b, t*128+s', c*128+p]
    encTt = nc.dram_tensor(
        "encTt", [bpc, s // P, P, h], BF16, kind="ExternalInput"
    ).ap()
    # dec_hidden repacked host-side into columns: deccol[p, b*nr+c] = dec[b, c*128+p]
    deccol = nc.dram_tensor("deccol", [P, bpc * nr], BF16, kind="ExternalInput").ap()
    mask = nc.dram_tensor("mask", [bpc, s], F32, kind="ExternalInput").ap()
    cov = nc.dram_tensor("cov", [bpc, s], F32, kind="ExternalInput").ap()
    nk6 = bpc + 2  # rank of the fused update: bpc dec rows + W_c + b_s
    # lc6[b] = [onehot(b); cov[b]; ones] — stationary side of the rank-(bpc+2)
    # update that folds dec_feature + coverage*W_c + b_s into the feat PSUM
    lc6 = nc.dram_tensor("lc6", [bpc, nk6, s], BF16, kind="ExternalInput").ap()
    # wb2 = [W_c[:,0]; b_s] — moving side rows 1-2 of the rank-3 update
    wb2 = nc.dram_tensor("wb2", [2, h], BF16, kind="ExternalInput").ap()
    whT = nc.dram_tensor("whT", [h, h], BF16, kind="ExternalInput").ap()
    wsT = nc.dram_tensor("wsT", [h, h], BF16, kind="ExternalInput").ap()
    v = nc.dram_tensor("v", [h], F32, kind="ExternalInput").ap()

    octx = nc.dram_tensor("octx", [bpc, h], F32, kind="ExternalOutput").ap()
    odist = nc.dram_tensor("odist", [bpc, s], F32, kind="ExternalOutput").ap()
    oncov = nc.dram_tensor("oncov", [bpc, s], F32, kind="ExternalOutput").ap()

    with TileContext(nc) as tc:
        with (
            tc.tile_pool(name="const", bufs=1) as cpool,
            tc.tile_pool(name="enc", bufs=5) as epool,
            tc.tile_pool(name="encT", bufs=3) as etpool,
            tc.tile_pool(name="tanh", bufs=2) as thpool,
            tc.tile_pool(name="scratch", bufs=2) as scpool,
            tc.tile_pool(name="small", bufs=2) as smpool,
            tc.tile_pool(name="psf", bufs=2, space="PSUM") as psf,
            tc.tile_pool(name="pst", bufs=2, space="PSUM") as pst,
            tc.tile_pool(name="psc", bufs=1, space="PSUM") as psc,
        ):
            # ---- constants ----
            ident_b = cpool.tile([P, P], BF16)
            make_identity(nc, ident_b)
            ident_f = cpool.tile([P, P], F32)
            make_identity(nc, ident_f)
            # HAM warmup: dependency-free junk matmuls keep the PE busy
            # from t~1us through the initial weight DMAs so the clock gate
            # is at 8/8 when real work starts (values are never read)
            junk = cpool.tile([P, P], BF16)
            nc.vector.memset(junk, 0.0)
            for j in range(48):
                jp = pst.tile([P, P], F32, tag="tp", name="jp")
                nc.tensor.matmul(
                    jp, lhsT=junk, rhs=junk, start=True, stop=True,
                    skip_group_check=True,
                )

            def setup(b, st=None):
                st = st if st is not None else {}
                nc.sync.dma_start(out=lcp[0:nk6, :], in_=lc6[b])
                cov_nat = smpool.tile([nt, P], F32, name="cov_nat")
                nc.sync.dma_start(
                    out=cov_nat, in_=cov[b].rearrange("(t p) -> t p", p=P)
                )
                mask_nat = smpool.tile([nt, P], F32, name="mask_nat")
                nc.sync.dma_start(
                    out=mask_nat, in_=mask[b].rearrange("(t p) -> t p", p=P)
                )
                mt_ps = pst.tile([P, 256], F32, tag="tp", name="mt_ps")
                nc.tensor.transpose(mt_ps[:, :nt], mask_nat, ident_f[:nt, :nt])
                maskT = smpool.tile([P, nt], F32, name="maskT")
                nc.vector.tensor_copy(maskT, mt_ps[:, :nt])

                st["cov_nat"] = cov_nat
                st["maskT"] = maskT
                st["p_all"] = smpool.tile([P, nt], BF16, name="p_all")
                st["ctx_ps"] = psc.tile([1, h], F32, name="ctx_ps")
                st.setdefault("enc_t", [None] * nt)
                st["et"] = [None] * nt
                return st

            def epilogue(b, st, t):
                # exp(e_t + mask) -> p column t; then accumulate ctx.
                nc.scalar.activation(
                    st["p_all"][:, t : t + 1],
                    st["et"][t],
                    AF.Exp,
                    bias=st["maskT"][:, t : t + 1],
                    scale=1.0,
                )
                for ho in range(nho):
                    hsl = slice(ho * 512, (ho + 1) * 512)
                    nc.tensor.matmul(
                        st["ctx_ps"][0:1, hsl],
                        lhsT=st["p_all"][:, t : t + 1],
                        rhs=st["enc_t"][t][:, hsl],
                        start=(t == 0),
                        stop=(t == nt - 1),
                        skip_group_check=True,
                    )

            def load_enc(b, st, t):
                st["enc_t"][t] = epool.tile([P, h], BF16, tag="enc_t", name="enc_t")
                nc.sync.dma_start(
                    out=st["enc_t"][t], in_=enc[b, t * P : (t + 1) * P, :]
                )

            def load_encT(b, st, t):
                # h-contraction layout, prepacked host-side
                encT = etpool.tile([P, h], BF16, tag="encT", name="encT")
                nc.sync.dma_start(out=encT, in_=encTt[b, t])
                st["encT"][t] = encT

            def vdot(b, st, t):
                # e_t = sum_h tanh(feat) * V — runs one iteration behind so
                # the encT copies always lead the DVE stream.
                prod = scpool.tile([P, h], F32, tag="prod", name="prod")
                st["et"][t] = smpool.tile([P, 1], F32, tag="et", name="et")
                nc.vector.tensor_mul(prod, st["tanh"][t], v_bc)
                nc.vector.reduce_sum(st["et"][t], prod, axis=mybir.AxisListType.X)

            def sloop(b, st):
                for t in range(nt):
                    if t + 2 < nt:
                        load_enc(b, st, t + 2)
                        load_encT(b, st, t + 2)
                    if t > 0:
                        vdot(b, st, t - 1)
                    feat = psf.tile([P, h], F32, tag="feat", name="feat")
                    for ho in range(nho):
                        hsl = slice(ho * 512, (ho + 1) * 512)
                        for c in range(nck):
                            nc.tensor.matmul(
                                feat[:, hsl],
                                lhsT=st["encT"][t][:, c * P : (c + 1) * P],
                                rhs=w_c[c][:, hsl],
                                start=(c == 0),
                                stop=False,
                            )
                        nc.tensor.matmul(
                            feat[:, hsl],
                            lhsT=lcp[:, t * P : (t + 1) * P],
                            rhs=rxp[:, hsl],
                            start=False,
                            stop=True,
                        )
                    if t > 1:
                        epilogue(b, st, t - 2)
                    tanh_t = thpool.tile([P, h], F32, tag="tanh", name="tanh_t")
                    nc.scalar.activation(tanh_t, feat, AF.Tanh)
                    st["tanh"][t] = tanh_t

            def finish(b, st):
                vdot(b, st, nt - 1)
                epilogue(b, st, nt - 2)
                epilogue(b, st, nt - 1)

            def tail(b, st):
                # total, normalize, outputs
                prow = smpool.tile([P, 1], F32, tag="prow", name="prow")
                nc.vector.reduce_sum(prow, st["p_all"], axis=mybir.AxisListType.X)
                # [nt,1] of the total: ones[128,nt].T @ prow replicates the
                # partition-sum into every output partition (fp32, exact).
                tot_ps = pst.tile([P, 512], F32, tag="tp", name="tot_ps")
                nc.tensor.matmul(
                    tot_ps[:nt, 0:1],
                    lhsT=nt_ones,
                    rhs=prow,
                    start=True,
                    stop=True,
                    skip_group_check=True,
                )
                inv128 = smpool.tile([nt, 1], F32, tag="inv128", name="inv128")
                nc.vector.reciprocal(inv128, tot_ps[:nt, 0:1])

                pt_ps = pst.tile([P, P], BF16, tag="tp", name="pt_ps")
                nc.tensor.transpose(pt_ps[:nt, :P], st["p_all"], ident_b)
                dist_nat = smpool.tile([nt, P], F32, tag="dist_nat", name="dist_nat")
                nc.vector.tensor_scalar_mul(
                    dist_nat, pt_ps[:nt, :P], inv128[:nt, :]
                )
                ncov_nat = smpool.tile([nt, P], F32, tag="ncov_nat", name="ncov_nat")
                nc.vector.tensor_add(ncov_nat, st["cov_nat"], dist_nat)
                ctx_sb = smpool.tile([1, h], F32, tag="ctx_sb", name="ctx_sb")
                nc.vector.tensor_scalar_mul(ctx_sb, st["ctx_ps"], inv128[0:1, :])

                nc.sync.dma_start(
                    out=odist[b].rearrange("(t p) -> t p", p=P), in_=dist_nat
                )
                nc.sync.dma_start(
                    out=oncov[b].rearrange("(t p) -> t p", p=P), in_=ncov_nat
                )
                nc.sync.dma_start(
                    out=octx[b].rearrange("(o n) -> o n", o=1), in_=ctx_sb
                )

            # zero-padded stationary side of the fused update, rows
            # 0..nk6-1 refreshed per batch
            lcp = cpool.tile([P, s], BF16)
            nc.vector.memset(lcp, 0.0)

            # ---- first-wave DMAs: the tensors the PE needs first (first
            # encT tile, weights, dec columns) get the first queue slots ----
            st0 = {"enc_t": [None] * nt, "encT": [None] * nt, "tanh": [None] * nt}
            encT00 = etpool.tile([P, h], BF16, tag="encT", name="encT00")
            for q in range(4):
                qs = slice(q * (h // 4), (q + 1) * (h // 4))
                nc.sync.dma_start(out=encT00[:, qs], in_=encTt[0, 0][:, qs])
            st0["encT"][0] = encT00
            # per-chunk weight tiles so the first matmuls only wait on the
            # first 128 rows, not the whole 2 MB weight DMA
            w_c = []
            for c in range(nck):
                wt = cpool.tile([P, h], BF16, name=f"w_c{c}")
                nc.sync.dma_start(out=wt, in_=whT[c * P : (c + 1) * P, :])
                w_c.append(wt)
            encT01 = etpool.tile([P, h], BF16, tag="encT", name="encT01")
            for q in range(2):
                qs = slice(q * (h // 2), (q + 1) * (h // 2))
                nc.sync.dma_start(out=encT01[:, qs], in_=encTt[0, 1][:, qs])
            st0["encT"][1] = encT01
            ws_c = []
            for c in range(nck):
                wt = cpool.tile([P, h], BF16, name=f"ws_c{c}")
                nc.sync.dma_start(out=wt, in_=wsT[c * P : (c + 1) * P, :])
                ws_c.append(wt)
            decall = cpool.tile([P, bpc * nr], BF16)
            nc.sync.dma_start(out=decall, in_=deccol)
            load_enc(0, st0, 0)
            load_enc(0, st0, 1)
            setup(0, st0)
            v_bc = cpool.tile([P, h], F32)
            nc.sync.dma_start(out=v_bc, in_=v.partition_broadcast(P))
            nt_ones = cpool.tile([P, nt], F32)
            nc.vector.memset(nt_ones, 1.0)

            # rxp = [dec_features(all 4 batches); W_c; b_s; zeros...] — the
            # moving side of the per-tile fused update, zero-padded to K=128
            # so the matmul takes the fast (FWL) weight-load path.  Rows 0-3
            # computed on device with one M=4 matmul group per h_out half.
            rxp = cpool.tile([P, h], BF16)
            nc.vector.memset(rxp, 0.0)
            nc.sync.dma_start(out=rxp[bpc : bpc + 2, :], in_=wb2)
            for ho in range(nho):
                hsl = slice(ho * 512, (ho + 1) * 512)
                dps = pst.tile([bpc, 512], F32, tag="tp", name="dps")
                for c in range(nck):
                    nc.tensor.matmul(
                        dps,
                        lhsT=decall[:, c::nr],
                        rhs=ws_c[c][:, hsl],
                        start=(c == 0),
                        stop=(c == nck - 1),
                    )
                nc.vector.tensor_copy(rxp[0:bpc, hsl], dps)
            # Batch-tail software pipeline: batch b's last epilogue and tail
            # are emitted after batch b+1's setup and first transposes, so
            # the PE fills the softmax-chain wait with useful work.
            states = {0: st0}
            for b in range(bpc):
                st = states[b]
                if b > 0:
                    finish(b - 1, states[b - 1])
                    tail(b - 1, states[b - 1])
                sloop(b, st)
                if b + 1 < bpc:
                    # next batch: enc prefetch + small-tensor DMAs
                    nx = {
                        "enc_t": [None] * nt,
                        "encT": [None] * nt,
                        "tanh": [None] * nt,
                    }
                    states[b + 1] = nx
                    load_encT(b + 1, nx, 0)
                    load_enc(b + 1, nx, 0)
                    load_encT(b + 1, nx, 1)
                    load_enc(b + 1, nx, 1)
                    setup(b + 1, nx)
            finish(bpc - 1, states[bpc - 1])
            tail(bpc - 1, states[bpc - 1])
    _cap_waits(nc)
    return nc


def make_nc(bpc=B // N_CORES, s=S, h=H):
    nc = bass.Bass("TRN2", target_bir_lowering=False, debug=False, num_devices=1)
    return build(nc, bpc, s, h)


def make_in_maps(enc_output, dec_hidden, enc_padding_mask, coverage, W_h, W_s, b_s, W_c, V):
    import ml_dtypes

    bf16 = ml_dtypes.bfloat16
    enc_output = np.asarray(enc_output, dtype=np.float32).astype(bf16)
    dec_hidden = np.asarray(dec_hidden, dtype=np.float32).astype(bf16)
    enc_padding_mask = np.ascontiguousarray(np.asarray(enc_padding_mask, dtype=np.float32))
    coverage = np.ascontiguousarray(np.asarray(coverage, dtype=np.float32))
    whT = np.ascontiguousarray(np.asarray(W_h, dtype=np.float32).T.astype(bf16))
    wsT = np.ascontiguousarray(np.asarray(W_s, dtype=np.float32).T.astype(bf16))
    bs = np.asarray(b_s, dtype=np.float32).astype(bf16)
    wc = np.asarray(W_c, dtype=np.float32)[:, 0].astype(bf16)
    v = np.ascontiguousarray(np.asarray(V, dtype=np.float32)[0])
    wb2 = np.ascontiguousarray(np.stack([wc, bs], axis=0))

    nb, s = coverage.shape
    h = v.shape[0]
    nr = h // P
    bpc = nb // N_CORES
    in_maps = []
    for i in range(N_CORES):
        sl = slice(i * bpc, (i + 1) * bpc)
        dec_i = dec_hidden[sl]  # [bpc, h]
        deccol = np.ascontiguousarray(
            dec_i.reshape(bpc, nr, P).transpose(2, 0, 1).reshape(P, bpc * nr)
        )
        lc6 = np.zeros((bpc, bpc + 2, s), dtype=bf16)
        for bb in range(bpc):
            lc6[bb, bb, :] = 1.0
        lc6[:, bpc, :] = coverage[sl].astype(bf16)
        lc6[:, bpc + 1, :] = 1.0
        in_maps.append(
            {
                "enc": np.ascontiguousarray(enc_output[sl]),
                "encTt": np.ascontiguousarray(
                    enc_output[sl]
                    .reshape(bpc, s // P, P, h // P, P)
                    .transpose(0, 1, 4, 3, 2)
                    .reshape(bpc, s // P, P, h)
                ),
                "deccol": deccol,
                "mask": enc_padding_mask[sl],
                "cov": coverage[sl],
                "lc6": np.ascontiguousarray(lc6),
                "wb2": wb2,
                "whT": whT,
                "wsT": wsT,
                "v": v,
            }
        )
    return in_maps, bpc


def run(in_maps, nc=None, **kw):
    from concourse.bass_utils import run_bass_kernel_spmd

    if nc is None:
        nc = make_nc()
    return run_bass_kernel_spmd(nc, in_maps, core_ids=list(range(N_CORES)), **kw)


def kernel(**inputs):
    in_maps, bpc = make_in_maps(**inputs)
    res = run(in_maps)
    ctx = np.concatenate([res.results[i]["octx"] for i in range(N_CORES)], axis=0)
    dist = np.concatenate([res.results[i]["odist"] for i in range(N_CORES)], axis=0)
    ncov = np.concatenate([res.results[i]["oncov"] for i in range(N_CORES)], axis=0)
    return ctx, dist, ncov


# revision 45
# speedup vs baseline: 1.0091x; 1.0026x over previous
"""Bass/Tile TRN2 kernel for the coverage-attention module.

Math (per batch b):
    enc_feature = enc_output[b] @ W_h.T                    [S, H]
    dec_feature = W_s @ dec_hidden[b] + b_s                [H]
    feat        = enc_feature + dec_feature + cov[b,:,None]*W_c[:,0]
    e_t         = tanh(feat) @ V[0]                        [S]
    dist        = softmax(e_t + mask[b])                   [S]
    ctx         = dist @ enc_output[b]                     [H]
    new_cov     = cov[b] + dist

Strategy: data-parallel over batch across 8 cores (4 batches/core),
weights replicated.  Matmuls run in bf16 (fp32 matmul is 4x slower on
the PE; bf16 with K=1024 fp32 accumulation keeps relative error around
2.4e-3, well under the 2e-2 gate).  Per batch the sequence runs in 16
s-tiles of 128 rows, single pass over enc_output:
  - The h-contraction ("transposed") layout of each enc tile is packed
    host-side (pure layout prep, like the weight transposes) and DMA'd
    directly, so the PE spends no cycles transposing.
  - PE: 16 N=512 main matmuls per s-tile; one zero-padded K=128 rank
    update per h_out half folds dec_feature + coverage*W_c + b_s into
    the same PSUM accumulation (zero-padding keeps the fast FWL weight
    path: ~225ns vs ~317ns per matmul at K<32); the context vector
    accumulates via [K=128,M=1]x[K=128,N=512] matmuls against exp(e_t).
  - ACT: tanh (PSUM->SBUF, fp32) and exp (the padding mask enters as
    the per-partition bias of the exp activation).
  - DVE: V-dot as tensor_mul + reduce_sum (fp32).
Softmax is computed without max subtraction (|e_t| <= sum|V| <= 16 so
exp cannot overflow in fp32), which makes it single-pass: p-columns
feed the context matmul two iterations behind the main loop;
normalization by 1/total happens at the batch tail, where a
[128,nt]-ones fp32 matmul replicates the partition-sum total across
partitions (exact, no extra rounding).

Scheduling: dependency-free junk matmuls warm the PE clock gate (HAM)
during the initial weight DMAs; the first encT tile is split across 4
DMA queues; batch b's last epilogue + tail are emitted after batch
b+1's prefetch/setup so the PE fills the softmax-chain wait; the
whole kernel is straight-line code (no loops) scheduled by Tile.

This walrus build rejects >1 sync wait per instruction (2 for
EventSemaphore): _cap_waits() post-processes the scheduled program,
spilling excess waits onto single-wait EventSemaphore instructions on
the same engine.
"""

import numpy as np

import concourse.bass as bass
import concourse.mybir as mybir
from concourse.masks import make_identity
from concourse.tile import TileContext

F32 = mybir.dt.float32
BF16 = mybir.dt.bfloat16
AF = mybir.ActivationFunctionType
ALU = mybir.AluOpType

N_CORES = 8
B, S, H = 32, 2048, 1024
P = 128


def _cap_waits(nc):
    """Cap sync waits per instruction for this container's walrus build.

    The walrus here rejects instructions carrying more than 1 sync wait
    (2 for EventSemaphore) with "Too many sync wait commands", while the
    Tile scheduler freely attaches one wait per dependency.  Move excess
    waits onto InstEventSemaphore instructions inserted just before the
    owner on the same engine — the engine executes its stream in order,
    so the conjunction of waits is preserved.
    """
    import bass_rust

    for f in nc.m.functions:
        for blk in f.blocks:
            insts = blk.instructions
            new_list = []
            changed = False
            for ins in insts:
                si = ins.sync_info
                waits = list(si.on_wait) if si is not None and si.on_wait else []
                cap = 2 if isinstance(ins, mybir.InstEventSemaphore) else 1
                if len(waits) > cap:
                    changed = True
                    extra, keep = waits[:-cap], waits[-cap:]
                    for j in range(0, len(extra), 2):
                        ev = mybir.InstEventSemaphore(
                            name=f"I-{nc.next_id()}",
                            engine=ins.engine,
                            ins=[],
                            outs=[],
                            sync_info=bass_rust.SyncInfo(
                                on_wait=extra[j : j + 2], on_update=[]
                            ),
                        )
                        new_list.append(ev)
                    ins.sync_info = bass_rust.SyncInfo(
                        on_wait=keep,
                        on_update=list(si.on_update) if si.on_update else [],
                    )
                new_list.append(ins)
            if changed:
                blk.instructions = new_list


def build(nc, bpc, s, h):
    """Emit the per-core program. bpc batches of [s, h] per core."""
    nt = s // P          # s-tiles per batch
    nck = h // P         # h_in chunks of 128
    nho = h // 512       # h_out chunks of 512
    nr = h // P          # rows per batch in the dec reshape

    enc = nc.dram_tensor("enc", [bpc, s, h], BF16, kind="ExternalInput").ap()
    # per-s-tile transposed enc: encTt[b, t, p, c*128+s'] = enc[b, t*128+s', c*128+p]
    encTt = nc.dram_tensor(
        "encTt", [bpc, s // P, P, h], BF16, kind="ExternalInput"
    ).ap()
    # dec_hidden repacked host-side into columns: deccol[p, b*nr+c] = dec[b, c*128+p]
    deccol = nc.dram_tensor("deccol", [P, bpc * nr], BF16, kind="ExternalInput").ap()
    mask = nc.dram_tensor("mask", [bpc, s], F32, kind="ExternalInput").ap()
    cov = nc.dram_tensor("cov", [bpc, s], F32, kind="ExternalInput").ap()
    nk6 = bpc + 2  # rank of the fused update: bpc dec rows + W_c + b_s
    # lc6[b] = [onehot(b); cov[b]; ones] — stationary side of the rank-(bpc+2)
    # update that folds dec_feature + coverage*W_c + b_s into the feat PSUM
    lc6 = nc.dram_tensor("lc6", [bpc, nk6, s], BF16, kind="ExternalInput").ap()
    # wb2 = [W_c[:,0]; b_s] — moving side rows 1-2 of the rank-3 update
    wb2 = nc.dram_tensor("wb2", [2, h], BF16, kind="ExternalInput").ap()
    whT = nc.dram_tensor("whT", [h, h], BF16, kind="ExternalInput").ap()
    wsT = nc.dram_tensor("wsT", [h, h], BF16, kind="ExternalInput").ap()
    v = nc.dram_tensor("v", [h], F32, kind="ExternalInput").ap()

    octx = nc.dram_tensor("octx", [bpc, h], F32, kind="ExternalOutput").ap()
    odist = nc.dram_tensor("odist", [bpc, s], F32, kind="ExternalOutput").ap()
    oncov = nc.dram_tensor("oncov", [bpc, s], F32, kind="ExternalOutput").ap()

    with TileContext(nc) as tc:
        with (
            tc.tile_pool(name="const", bufs=1) as cpool,
            tc.tile_pool(name="enc", bufs=6) as epool,
            tc.tile_pool(name="encT", bufs=4) as etpool,
            tc.tile_pool(name="tanh", bufs=2) as thpool,
            tc.tile_pool(name="scratch", bufs=2) as scpool,
            tc.tile_pool(name="small", bufs=2) as smpool,
            tc.tile_pool(name="psf", bufs=2, space="PSUM") as psf,
            tc.tile_pool(name="pst", bufs=2, space="PSUM") as pst,
            tc.tile_pool(name="psc", bufs=1, space="PSUM") as psc,
        ):
            # ---- constants ----
            ident_b = cpool.tile([P, P], BF16)
            make_identity(nc, ident_b)
            ident_f = cpool.tile([P, P], F32)
            make_identity(nc, ident_f)
            # HAM warmup: dependency-free junk matmuls keep the PE busy
            # from t~1us through the initial weight DMAs so the clock gate
            # is at 8/8 when real work starts (values are never read)
            junk = cpool.tile([P, P], BF16)
            nc.vector.memset(junk, 0.0)
            for j in range(48):
                jp = pst.tile([P, P], F32, tag="tp", name="jp")
                nc.tensor.matmul(
                    jp, lhsT=junk, rhs=junk, start=True, stop=True,
                    skip_group_check=True,
                )

            def setup(b, st=None):
                st = st if st is not None else {}
                nc.sync.dma_start(out=lcp[0:nk6, :], in_=lc6[b])
                cov_nat = smpool.tile([nt, P], F32, name="cov_nat")
                nc.sync.dma_start(
                    out=cov_nat, in_=cov[b].rearrange("(t p) -> t p", p=P)
                )
                mask_nat = smpool.tile([nt, P], F32, name="mask_nat")
                nc.sync.dma_start(
                    out=mask_nat, in_=mask[b].rearrange("(t p) -> t p", p=P)
                )
                mt_ps = pst.tile([P, 256], F32, tag="tp", name="mt_ps")
                nc.tensor.transpose(mt_ps[:, :nt], mask_nat, ident_f[:nt, :nt])
                maskT = smpool.tile([P, nt], F32, name="maskT")
                nc.vector.tensor_copy(maskT, mt_ps[:, :nt])

                st["cov_nat"] = cov_nat
                st["maskT"] = maskT
                st["p_all"] = smpool.tile([P, nt], BF16, name="p_all")
                st["ctx_ps"] = psc.tile([1, h], F32, name="ctx_ps")
                st.setdefault("enc_t", [None] * nt)
                st["et"] = [None] * nt
                return st

            def epilogue(b, st, t):
                # exp(e_t + mask) -> p column t; then accumulate ctx.
                nc.scalar.activation(
                    st["p_all"][:, t : t + 1],
                    st["et"][t],
                    AF.Exp,
                    bias=st["maskT"][:, t : t + 1],
                    scale=1.0,
                )
                for ho in range(nho):
                    hsl = slice(ho * 512, (ho + 1) * 512)
                    nc.tensor.matmul(
                        st["ctx_ps"][0:1, hsl],
                        lhsT=st["p_all"][:, t : t + 1],
                        rhs=st["enc_t"][t][:, hsl],
                        start=(t == 0),
                        stop=(t == nt - 1),
                        skip_group_check=True,
                    )

            def load_enc(b, st, t):
                st["enc_t"][t] = epool.tile([P, h], BF16, tag="enc_t", name="enc_t")
                nc.sync.dma_start(
                    out=st["enc_t"][t], in_=enc[b, t * P : (t + 1) * P, :]
                )

            def load_encT(b, st, t):
                # h-contraction layout, prepacked host-side
                encT = etpool.tile([P, h], BF16, tag="encT", name="encT")
                nc.sync.dma_start(out=encT, in_=encTt[b, t])
                st["encT"][t] = encT

            def vdot(b, st, t):
                # e_t = sum_h tanh(feat) * V — runs one iteration behind so
                # the encT copies always lead the DVE stream.
                prod = scpool.tile([P, h], F32, tag="prod", name="prod")
                st["et"][t] = smpool.tile([P, 1], F32, tag="et", name="et")
                nc.vector.tensor_mul(prod, st["tanh"][t], v_bc)
                nc.vector.reduce_sum(st["et"][t], prod, axis=mybir.AxisListType.X)

            def sloop(b, st):
                for t in range(nt):
                    if t == 0 and st["enc_t"][2] is None:
                        load_enc(b, st, 2)
                        load_encT(b, st, 2)
                    if t + 3 < nt:
                        load_enc(b, st, t + 3)
                        load_encT(b, st, t + 3)
                    if t > 0:
                        vdot(b, st, t - 1)
                    feat = psf.tile([P, h], F32, tag="feat", name="feat")
                    for ho in range(nho):
                        hsl = slice(ho * 512, (ho + 1) * 512)
                        for c in range(nck):
                            nc.tensor.matmul(
                                feat[:, hsl],
                                lhsT=st["encT"][t][:, c * P : (c + 1) * P],
                                rhs=w_c[c][:, hsl],
                                start=(c == 0),
                                stop=False,
                            )
                        nc.tensor.matmul(
                            feat[:, hsl],
                            lhsT=lcp[:, t * P : (t + 1) * P],
                            rhs=rxp[:, hsl],
                            start=False,
                            stop=True,
                        )
                    if t > 1:
                        epilogue(b, st, t - 2)
                    tanh_t = thpool.tile([P, h], F32, tag="tanh", name="tanh_t")
                    nc.scalar.activation(tanh_t, feat, AF.Tanh)
                    st["tanh"][t] = tanh_t

            def finish(b, st):
                vdot(b, st, nt - 1)
                epilogue(b, st, nt - 2)
                epilogue(b, st, nt - 1)

            def tail(b, st):
                # total, normalize, outputs
                prow = smpool.tile([P, 1], F32, tag="prow", name="prow")
                nc.vector.reduce_sum(prow, st["p_all"], axis=mybir.AxisListType.X)
                # [nt,1] of the total: ones[128,nt].T @ prow replicates the
                # partition-sum into every output partition (fp32, exact).
                tot_ps = pst.tile([P, 512], F32, tag="tp", name="tot_ps")
                nc.tensor.matmul(
                    tot_ps[:nt, 0:1],
                    lhsT=nt_ones,
                    rhs=prow,
                    start=True,
                    stop=True,
                    skip_group_check=True,
                )
                inv128 = smpool.tile([nt, 1], F32, tag="inv128", name="inv128")
                nc.vector.reciprocal(inv128, tot_ps[:nt, 0:1])

                pt_ps = pst.tile([P, P], BF16, tag="tp", name="pt_ps")
                nc.tensor.transpose(pt_ps[:nt, :P], st["p_all"], ident_b)
                dist_nat = smpool.tile([nt, P], F32, tag="dist_nat", name="dist_nat")
                nc.vector.tensor_scalar_mul(
                    dist_nat, pt_ps[:nt, :P], inv128[:nt, :]
                )
                ncov_nat = smpool.tile([nt, P], F32, tag="ncov_nat", name="ncov_nat")
                nc.vector.tensor_add(ncov_nat, st["cov_nat"], dist_nat)
                ctx_sb = smpool.tile([1, h], F32, tag="ctx_sb", name="ctx_sb")
                nc.vector.tensor_scalar_mul(ctx_sb, st["ctx_ps"], inv128[0:1, :])

                nc.sync.dma_start(
                    out=odist[b].rearrange("(t p) -> t p", p=P), in_=dist_nat
                )
                nc.sync.dma_start(
                    out=oncov[b].rearrange("(t p) -> t p", p=P), in_=ncov_nat
                )
                nc.sync.dma_start(
                    out=octx[b].rearrange("(o n) -> o n", o=1), in_=ctx_sb
                )

            # zero-padded stationary side of the fused update, rows
            # 0..nk6-1 refreshed per batch
            lcp = cpool.tile([P, s], BF16)
            nc.vector.memset(lcp, 0.0)

            # ---- first-wave DMAs: the tensors the PE needs first (first
            # encT tile, weights, dec columns) get the first queue slots ----
            st0 = {"enc_t": [None] * nt, "encT": [None] * nt, "tanh": [None] * nt}
            encT00 = etpool.tile([P, h], BF16, tag="encT", name="encT00")
            for q in range(4):
                qs = slice(q * (h // 4), (q + 1) * (h // 4))
                nc.sync.dma_start(out=encT00[:, qs], in_=encTt[0, 0][:, qs])
            st0["encT"][0] = encT00
            # per-chunk weight tiles so the first matmuls only wait on the
            # first 128 rows, not the whole 2 MB weight DMA
            w_c = [
                cpool.tile([P, h], BF16, name=f"w_c{c}") for c in range(nck)
            ]
            ws_c = [
                cpool.tile([P, h], BF16, name=f"ws_c{c}") for c in range(nck)
            ]
            for c in range(nck // 2):
                nc.sync.dma_start(out=w_c[c], in_=whT[c * P : (c + 1) * P, :])
            for c in range(nck // 2):
                nc.sync.dma_start(out=ws_c[c], in_=wsT[c * P : (c + 1) * P, :])
            decall = cpool.tile([P, bpc * nr], BF16)
            nc.sync.dma_start(out=decall, in_=deccol)
            for c in range(nck // 2, nck):
                nc.sync.dma_start(out=w_c[c], in_=whT[c * P : (c + 1) * P, :])
            for c in range(nck // 2, nck):
                nc.sync.dma_start(out=ws_c[c], in_=wsT[c * P : (c + 1) * P, :])
            encT01 = etpool.tile([P, h], BF16, tag="encT", name="encT01")
            for q in range(2):
                qs = slice(q * (h // 2), (q + 1) * (h // 2))
                nc.sync.dma_start(out=encT01[:, qs], in_=encTt[0, 1][:, qs])
            st0["encT"][1] = encT01
            load_enc(0, st0, 0)
            load_enc(0, st0, 1)
            setup(0, st0)
            v_bc = cpool.tile([P, h], F32)
            nc.sync.dma_start(out=v_bc, in_=v.partition_broadcast(P))
            nt_ones = cpool.tile([P, nt], F32)
            nc.vector.memset(nt_ones, 1.0)

            # rxp = [dec_features(all 4 batches); W_c; b_s; zeros...] — the
            # moving side of the per-tile fused update, zero-padded to K=128
            # so the matmul takes the fast (FWL) weight-load path.  Rows 0-3
            # computed on device with one M=4 matmul group per h_out half.
            rxp = cpool.tile([P, h], BF16)
            nc.vector.memset(rxp, 0.0)
            nc.sync.dma_start(out=rxp[bpc : bpc + 2, :], in_=wb2)
            for ho in range(nho):
                hsl = slice(ho * 512, (ho + 1) * 512)
                dps = pst.tile([bpc, 512], F32, tag="tp", name="dps")
                for c in range(nck):
                    nc.tensor.matmul(
                        dps,
                        lhsT=decall[:, c::nr],
                        rhs=ws_c[c][:, hsl],
                        start=(c == 0),
                        stop=(c == nck - 1),
                    )
                nc.vector.tensor_copy(rxp[0:bpc, hsl], dps)
            # Batch-tail software pipeline: batch b's last epilogue and tail
            # are emitted after batch b+1's setup and first transposes, so
            # the PE fills the softmax-chain wait with useful work.
            states = {0: st0}
            for b in range(bpc):
                st = states[b]
                if b > 0:
                    finish(b - 1, states[b - 1])
                    tail(b - 1, states[b - 1])
                sloop(b, st)
                if b + 1 < bpc:
                    # next batch: enc prefetch + small-tensor DMAs
                    nx = {
                        "enc_t": [None] * nt,
                        "encT": [None] * nt,
                        "tanh": [None] * nt,
                    }
                    states[b + 1] = nx
                    load_encT(b + 1, nx, 0)
                    load_enc(b + 1, nx, 0)
                    load_encT(b + 1, nx, 1)
                    load_enc(b + 1, nx, 1)
                    setup(b + 1, nx)
            finish(bpc - 1, states[bpc - 1])
            tail(bpc - 1, states[bpc - 1])
    _cap_waits(nc)
    return nc


def make_nc(bpc=B // N_CORES, s=S, h=H):
    nc = bass.Bass("TRN2", target_bir_lowering=False, debug=False, num_devices=1)
    return build(nc, bpc, s, h)


def make_in_maps(enc_output, dec_hidden, enc_padding_mask, coverage, W_h, W_s, b_s, W_c, V):
    import ml_dtypes

    bf16 = ml_dtypes.bfloat16
    enc_output = np.asarray(enc_output, dtype=np.float32).astype(bf16)
    dec_hidden = np.asarray(dec_hidden, dtype=np.float32).astype(bf16)
    enc_padding_mask = np.ascontiguousarray(np.asarray(enc_padding_mask, dtype=np.float32))
    coverage = np.ascontiguousarray(np.asarray(coverage, dtype=np.float32))
    whT = np.ascontiguousarray(np.asarray(W_h, dtype=np.float32).T.astype(bf16))
    wsT = np.ascontiguousarray(np.asarray(W_s, dtype=np.float32).T.astype(bf16))
    bs = np.asarray(b_s, dtype=np.float32).astype(bf16)
    wc = np.asarray(W_c, dtype=np.float32)[:, 0].astype(bf16)
    v = np.ascontiguousarray(np.asarray(V, dtype=np.float32)[0])
    wb2 = np.ascontiguousarray(np.stack([wc, bs], axis=0))

    nb, s = coverage.shape
    h = v.shape[0]
    nr = h // P
    bpc = nb // N_CORES
    in_maps = []
    for i in range(N_CORES):
        sl = slice(i * bpc, (i + 1) * bpc)
        dec_i = dec_hidden[sl]  # [bpc, h]
        deccol = np.ascontiguousarray(
            dec_i.reshape(bpc, nr, P).transpose(2, 0, 1).reshape(P, bpc * nr)
        )
        lc6 = np.zeros((bpc, bpc + 2, s), dtype=bf16)
        for bb in range(bpc):
            lc6[bb, bb, :] = 1.0
        lc6[:, bpc, :] = coverage[sl].astype(bf16)
        lc6[:, bpc + 1, :] = 1.0
        in_maps.append(
            {
                "enc": np.ascontiguousarray(enc_output[sl]),
                "encTt": np.ascontiguousarray(
                    enc_output[sl]
                    .reshape(bpc, s // P, P, h // P, P)
                    .transpose(0, 1, 4, 3, 2)
                    .reshape(bpc, s // P, P, h)
                ),
                "deccol": deccol,
                "mask": enc_padding_mask[sl],
                "cov": coverage[sl],
                "lc6": np.ascontiguousarray(lc6),
                "wb2": wb2,
                "whT": whT,
                "wsT": wsT,
                "v": v,
            }
        )
    return in_maps, bpc


def run(in_maps, nc=None, **kw):
    from concourse.bass_utils import run_bass_kernel_spmd

    if nc is None:
        nc = make_nc()
    return run_bass_kernel_spmd(nc, in_maps, core_ids=list(range(N_CORES)), **kw)


def kernel(**inputs):
    in_maps, bpc = make_in_maps(**inputs)
    res = run(in_maps)
    ctx = np.concatenate([res.results[i]["octx"] for i in range(N_CORES)], axis=0)
    dist = np.concatenate([res.results[i]["odist"] for i in range(N_CORES)], axis=0)
    ncov = np.concatenate([res.results[i]["oncov"] for i in range(N_CORES)], axis=0)
    return ctx, dist, ncov
